# revision 23
# baseline (speedup 1.0000x reference)
# Involution2d (K=7) Trainium2 kernel — 8-core SPMD, batch+spatial sharding.
#
# Sharding: 8 cores = (batch b in 0..3) x (H-half in 0..1); each core owns a
# [128, 32, 64] output block (2048 pixels, p = 64*h + w).
#
# Per-core algorithm (all-TensorE involution via a banded pixel->pixel matrix):
#   1. gen (bf16): 1x1 conv (BN folded) -> ReLU -> 1x1 conv, emitted directly
#      in pixel-major layout kermT[p, o] (16 matmuls of [33,128]^T @ [33,49];
#      bias rides an ones-row in the stationary operand).
#   2. GPSIMD local_scatter per 128-pixel tile mb: place the 49 kernel values
#      of pixel p at column q - 128*mb of A2T[p, :], where q = p + 192 +
#      64*di + dj is the flattened source pixel (38 rows x 64 cols q-space;
#      halo rows from the neighbor core, zeros at image edges). W-edge terms
#      get idx=-1 (skipped), which provably clips the window to 512 columns.
#   3. TensorE transposes each 128x128 block (bf16, 1 cyc/row) into per-q-tile
#      strips strip[kb] = A2[q, p-window], copied PSUM->SBUF in one 512-wide
#      copy per strip (DVE/ScalarE alternating).
#   4. involution: out[c, p] = sum_q xT[q, c] * A2[q, p] as 28 accumulating
#      bf16 matmuls over 512-column PSUM group tiles, kb-major. Only the
#      first matmul of a group uses start=True (clears the bank's
#      has_written bits); later matmuls overwrite on first element touch and
#      accumulate on repeats.
import numpy as np
import ml_dtypes

EPS = 1e-5
KK = 7
C = 128
H = 64
W = 64
B = 4
HH = 32            # rows per core
P = HH * W         # 2048 output pixels per core
NQT = 19           # q tiles: (HH + 6) * W / 128
NO = 50            # offset count padded to even (49 + 1 dummy)
AWIN = 512         # scatter window (4 q-tiles)

# packed-constants byte layout (per partition)
OFF_W1 = 0         # [128, 32] bf16      -> 64 B
OFF_ID = 64        # [128, 128] bf16     -> 256 B (transpose identity)
OFF_I2 = 320       # [128, 100] int16    -> 200 B (2-tile scatter table)
OFF_B1 = 520       # [32, 1] f32         -> 4 B
OFF_W2 = 524       # [33, 49] bf16       -> 98 B
NCB = 624

_STATE = {}

BF16 = ml_dtypes.bfloat16


def _build():
    import concourse.tile as tile
    from concourse import bacc, mybir

    f32 = mybir.dt.float32
    bf16 = mybir.dt.bfloat16
    i16 = mybir.dt.int16
    u8 = mybir.dt.uint8
    u32 = mybir.dt.uint32
    nc = bacc.Bacc("TRN2", target_bir_lowering=False, debug=False)

    XCHUNKS = (128, 384, 512, 512, 512)
    xcm_d = [
        nc.dram_tensor(f"xcm{i}", [C, n], bf16, kind="ExternalInput").ap()
        for i, n in enumerate(XCHUNKS)
    ]
    xtp_d = [
        nc.dram_tensor(f"xtp{i}", [128, n * 128], bf16, kind="ExternalInput").ap()
        for i, n in ((0, 10), (1, 9))
    ]
    cb_d = nc.dram_tensor("cb", [128, NCB], u8, kind="ExternalInput").ap()
    out_d = nc.dram_tensor("out", [C, P], bf16, kind="ExternalOutput").ap()

    with tile.TileContext(nc) as tc:
        with (
            tc.tile_pool(name="consts", bufs=1) as cpool,
            tc.tile_pool(name="pgen", bufs=1, space="PSUM") as pgen,
            tc.tile_pool(name="pkt", bufs=2, space="PSUM") as pkt,
            tc.tile_pool(name="ptp", bufs=3, space="PSUM") as ptp,
            tc.tile_pool(name="pout", bufs=2, space="PSUM") as pout,
        ):
            # --- input DMAs on both HWDGE queues (sync + scalar) ---
            cb = cpool.tile([128, NCB], u8, tag="cb")
            nc.scalar.dma_start(cb[:], cb_d)
            xcm = []
            for i, n in enumerate(XCHUNKS):
                t = cpool.tile([C, n], bf16, tag=f"xcm{i}", name=f"xcm{i}")
                nc.sync.dma_start(t[:], xcm_d[i])
                xcm.append(t)
            xtp = []
            for i, n in ((0, 10), (1, 9)):
                t = cpool.tile([128, n * 128], bf16, tag=f"xtp{i}", name=f"xtp{i}")
                nc.scalar.dma_start(t[:], xtp_d[i])
                xtp.append(t)

            w1sT = cb[:, OFF_W1:OFF_W1 + 64].bitcast(bf16)       # [128, 32]
            ident = cb[:, OFF_ID:OFF_ID + 256].bitcast(bf16)     # [128, 128]
            idxt2 = cb[:, OFF_I2:OFF_I2 + 200].bitcast(i16)      # [128, 100]
            b1f = cb[0:32, OFF_B1:OFF_B1 + 4].bitcast(f32)       # [32, 1]
            w2b = cb[0:33, OFF_W2:OFF_W2 + 98].bitcast(bf16)     # [33, 49]

            def xtp_tile(kb):
                return (xtp[0][:, kb * 128:(kb + 1) * 128] if kb < 10
                        else xtp[1][:, (kb - 10) * 128:(kb - 9) * 128])

            outsb = cpool.tile([C, P], bf16, tag="outsb")
            fb = cpool.tile([33, P], bf16, tag="fb")
            # ones row of fb, written as packed pairs of bf16(1.0)
            nc.vector.memset(fb[32:33, :].bitcast(u32), 0x3F803F80)
            kermT = cpool.tile([128, 16 * NO], bf16, tag="kermT")

            # ---- kernel generation (pixel-major kermT[p, o]) interleaved
            # with the GPSIMD banded-matrix scatters; a tiny first chunk gets
            # the scatter chain started as early as possible ----
            a2v = [None] * 16
            CHUNK_TILES = ((0,), (1, 2, 3), (4, 5, 6, 7), (8, 9, 10, 11),
                           (12, 13, 14, 15))
            CHUNK_BATCHES = (((0,),), ((1,), (2, 3)), ((4, 5), (6, 7)),
                             ((8, 9), (10, 11)), ((12, 13), (14,), (15,)))
            off = 0
            for ci, n in enumerate(XCHUNKS):
                fsl = slice(off, off + n)
                off += n
                f1 = pgen.tile([32, 512], f32, tag="f1")
                nc.tensor.matmul(f1[:, 0:n], w1sT, xcm[ci][:],
                                 start=True, stop=True)
                nc.scalar.activation(
                    fb[0:32, fsl], f1[:, 0:n],
                    mybir.ActivationFunctionType.Relu, bias=b1f,
                )
                for t in CHUNK_TILES[ci]:
                    kt = pkt.tile([128, 512], f32, tag="kt")
                    nc.tensor.matmul(
                        kt[:, 0:49], fb[:, 128 * t:128 * (t + 1)], w2b,
                        start=True, stop=True,
                    )
                    nc.vector.tensor_copy(kermT[:, t * NO:t * NO + 49],
                                          kt[:, 0:49])
                for mbs_b in CHUNK_BATCHES[ci]:
                    k0 = mbs_b[0]
                    nb = len(mbs_b)
                    ab = cpool.tile([128, nb * AWIN], bf16, name=f"a2b{k0}",
                                    tag=f"a2b{k0}")
                    nc.gpsimd.local_scatter(
                        ab[:], kermT[:, k0 * NO:(k0 + nb) * NO],
                        idxt2[:, 0:nb * NO],
                        channels=128, num_elems=nb * AWIN, num_idxs=nb * NO,
                    )
                    for j, mb in enumerate(mbs_b):
                        a2v[mb] = ab[:, j * AWIN:(j + 1) * AWIN]

            # ---- transpose blocks into strips + kb-major matmuls,
            # lagged one iteration so TensorE reaches each matmul only after
            # its strip copy has had a full iteration to complete ----
            po = {}
            strips = [None] * NQT

            def emit_mms(kb):
                base = 128 * (kb - 3)
                for g in range(max(0, (kb - 3) // 4), min(3, kb // 4) + 1):
                    first = g not in po
                    if first:
                        po[g] = pout.tile([C, 512], f32, name=f"po{g}", tag="po")
                    glo, ghi = 512 * g, 512 * (g + 1)
                    last = kb == min(4 * g + 6, NQT - 1)
                    lo = max(glo, base)
                    hi = min(ghi, 128 * (kb + 1) if kb <= 15 else 128 * kb)
                    if hi > lo:
                        nc.tensor.matmul(
                            po[g][:, lo - glo:hi - glo],
                            xtp_tile(kb),
                            strips[kb][:, lo - base:hi - base],
                            start=first, stop=last, skip_group_check=True,
                        )
                    if last:
                        nc.vector.tensor_copy(
                            outsb[:, glo:glo + 256], po[g][:, 0:256])
                        nc.scalar.copy(
                            outsb[:, glo + 256:ghi], po[g][:, 256:512])
                        eng = nc.sync if g % 2 == 0 else nc.scalar
                        eng.dma_start(out_d[:, glo:ghi], outsb[:, glo:ghi])

            for kb in range(NQT):
                mbs = [kb - j for j in range(3, -1, -1) if 0 <= kb - j <= 15]
                tp = ptp.tile([128, 1024], bf16, tag="tp")
                for mb in mbs:
                    pos = mb - (kb - 3)
                    nc.tensor.transpose(
                        tp[:, pos * 128:(pos + 1) * 128],
                        a2v[mb][:, (kb - mb) * 128:(kb - mb + 1) * 128],
                        ident,
                    )
                strip = cpool.tile([128, AWIN], bf16, name=f"st{kb % 4}",
                                   tag=f"st{kb % 4}")
                strips[kb] = strip
                lo_pos = mbs[0] - (kb - 3)
                hi_pos = mbs[-1] - (kb - 3) + 1
                mid_pos = (lo_pos + hi_pos + 1) // 2
                csl0 = slice(lo_pos * 128, mid_pos * 128)
                csl1 = slice(mid_pos * 128, hi_pos * 128)
                nc.vector.tensor_copy(strip[:, csl0], tp[:, csl0])
                if hi_pos > mid_pos:
                    nc.scalar.copy(strip[:, csl1], tp[:, csl1])
                if kb >= 1:
                    emit_mms(kb - 1)
            emit_mms(NQT - 1)

    nc.compile()
    return nc


def _get_nc():
    if "nc" not in _STATE:
        _STATE["nc"] = _build()
    return _STATE["nc"]


def _make_idx_table():
    p_loc = np.arange(128)[:, None]
    o = np.arange(49)[None, :]
    di = o // 7 - 3
    dj = o % 7 - 3
    w_of = p_loc % 64
    idx = p_loc + 192 + 64 * di + dj
    masked = (w_of + dj < 0) | (w_of + dj >= 64)
    idx = np.where(masked, -1, idx)
    tab = np.full((128, NO), -1, dtype=np.int16)
    tab[:, :49] = idx.astype(np.int16)
    return tab


def _host_prep(x, w1, b1, bn_gamma, bn_beta, bn_mean, bn_var, w2, b2):
    x = np.asarray(x, dtype=np.float32)
    scale = np.asarray(bn_gamma) / np.sqrt(np.asarray(bn_var) + EPS)
    w1s = (np.asarray(w1) * scale[:, None]).astype(np.float32)
    b1f = (np.asarray(b1) * scale + np.asarray(bn_beta)
           - np.asarray(bn_mean) * scale).astype(np.float32)
    w1sT = np.ascontiguousarray(w1s.T).astype(BF16)            # [128, 32]
    w2b = np.vstack([np.asarray(w2, np.float32).T,
                     np.asarray(b2, np.float32)[None, :]]).astype(BF16)  # [33, 49]
    idxt = _make_idx_table()                                   # [128, 50] i16
    ident = np.eye(128, dtype=np.float32).astype(BF16)

    cb = np.zeros((128, NCB), np.uint8)
    cb[:, OFF_W1:OFF_W1 + 64] = np.ascontiguousarray(w1sT).view(np.uint8)
    cb[:, OFF_ID:OFF_ID + 256] = np.ascontiguousarray(ident).view(np.uint8)
    idxt2 = np.concatenate(
        [np.where(idxt >= 0, idxt + AWIN * j, -1) for j in range(2)],
        axis=1).astype(np.int16)
    cb[:, OFF_I2:OFF_I2 + 200] = idxt2.view(np.uint8)
    cb[0:32, OFF_B1:OFF_B1 + 4] = np.ascontiguousarray(
        b1f[:, None]).view(np.uint8)
    cb[0:33, OFF_W2:OFF_W2 + 98] = np.ascontiguousarray(w2b).view(np.uint8)

    in_maps = []
    for core in range(8):
        b, half = divmod(core, 2)
        h0 = HH * half
        xcm = np.ascontiguousarray(
            x[b, :, h0:h0 + HH, :].reshape(C, P)).astype(BF16)
        # q-space: rows h0-3 .. h0+35 (zeros outside the image)
        xe = np.zeros((C, HH + 6, W), dtype=np.float32)
        lo = max(0, h0 - 3)
        hi = min(H, h0 + HH + 3)
        xe[:, lo - (h0 - 3):hi - (h0 - 3), :] = x[b, :, lo:hi, :]
        xq = xe.reshape(C, NQT * 128).T                        # [2432, 128]
        xtp = np.ascontiguousarray(
            xq.reshape(NQT, 128, 128).transpose(1, 0, 2).reshape(128, NQT * 128)
        ).astype(BF16)
        bnds = np.cumsum((0, 128, 384, 512, 512, 512))
        im = {f"xcm{i}": xcm[:, bnds[i]:bnds[i + 1]] for i in range(5)}
        im.update({"xtp0": xtp[:, :10 * 128], "xtp1": xtp[:, 10 * 128:],
                   "cb": cb})
        in_maps.append(im)
    return in_maps


def run(inputs: dict, trace: bool = False):
    from concourse.bass_utils import run_bass_kernel_spmd

    nc = _get_nc()
    in_maps = _host_prep(**inputs)
    res = run_bass_kernel_spmd(
        nc, in_maps, core_ids=list(range(8)), trace=trace,
    )
    out = np.zeros((B, C, H, W), dtype=np.float32)
    for core in range(8):
        b, half = divmod(core, 2)
        h0 = HH * half
        out[b, :, h0:h0 + HH, :] = (
            res.results[core]["out"].astype(np.float32).reshape(C, HH, W)
        )
    return out, res


def kernel(**inputs) -> np.ndarray:
    out, _ = run(inputs, trace=False)
    return out


# revision 24
# speedup vs baseline: 1.0433x; 1.0433x over previous
# Involution2d (K=7) Trainium2 kernel — 8-core SPMD, batch+spatial sharding.
#
# Sharding: 8 cores = (batch b in 0..3) x (H-half in 0..1); each core owns a
# [128, 32, 64] output block (2048 pixels, p = 64*h + w).
#
# Per-core algorithm (all-TensorE involution via a banded pixel->pixel matrix):
#   1. gen (bf16): 1x1 conv (BN folded) -> ReLU -> 1x1 conv, emitted directly
#      in pixel-major layout kermT[p, o] (16 matmuls of [33,128]^T @ [33,49];
#      bias rides an ones-row in the stationary operand).
#   2. GPSIMD local_scatter per 128-pixel tile mb: place the 49 kernel values
#      of pixel p at column q - 128*mb of A2T[p, :], where q = p + 192 +
#      64*di + dj is the flattened source pixel (38 rows x 64 cols q-space;
#      halo rows from the neighbor core, zeros at image edges). W-edge terms
#      get idx=-1 (skipped), which provably clips the window to 512 columns.
#   3. TensorE transposes each 128x128 block (bf16, 1 cyc/row) into per-q-tile
#      strips strip[kb] = A2[q, p-window], copied PSUM->SBUF in one 512-wide
#      copy per strip (DVE/ScalarE alternating).
#   4. involution: out[c, p] = sum_q xT[q, c] * A2[q, p] as 28 accumulating
#      bf16 matmuls over 512-column PSUM group tiles, kb-major. Only the
#      first matmul of a group uses start=True (clears the bank's
#      has_written bits); later matmuls overwrite on first element touch and
#      accumulate on repeats.
import numpy as np
import ml_dtypes

EPS = 1e-5
KK = 7
C = 128
H = 64
W = 64
B = 4
HH = 32            # rows per core
P = HH * W         # 2048 output pixels per core
NQT = 19           # q tiles: (HH + 6) * W / 128
NO = 50            # offset count padded to even (49 + 1 dummy)
AWIN = 512         # scatter window (4 q-tiles)

# packed-constants byte layout (per partition)
OFF_W1 = 0         # [128, 32] bf16      -> 64 B
OFF_ID = 64        # [128, 128] bf16     -> 256 B (transpose identity)
OFF_I2 = 320       # [128, 100] int16    -> 200 B (2-tile scatter table)
OFF_B1 = 520       # [32, 1] f32         -> 4 B
OFF_W2 = 524       # [33, 49] bf16       -> 98 B
NCB = 624

_STATE = {}

BF16 = ml_dtypes.bfloat16


def _build():
    import concourse.tile as tile
    from concourse import bacc, mybir

    f32 = mybir.dt.float32
    bf16 = mybir.dt.bfloat16
    i16 = mybir.dt.int16
    u8 = mybir.dt.uint8
    u32 = mybir.dt.uint32
    nc = bacc.Bacc("TRN2", target_bir_lowering=False, debug=False)

    XCHUNKS = (128, 384, 512, 512, 512)
    xcm_d = [
        nc.dram_tensor(f"xcm{i}", [C, n], bf16, kind="ExternalInput").ap()
        for i, n in enumerate(XCHUNKS)
    ]
    xtp_d = [
        nc.dram_tensor(f"xtp{i}", [128, n * 128], bf16, kind="ExternalInput").ap()
        for i, n in ((0, 10), (1, 9))
    ]
    cb_d = nc.dram_tensor("cb", [128, NCB], u8, kind="ExternalInput").ap()
    out_d = nc.dram_tensor("out", [C, P], bf16, kind="ExternalOutput").ap()

    with tile.TileContext(nc) as tc:
        with (
            tc.tile_pool(name="consts", bufs=1) as cpool,
            tc.tile_pool(name="pgen", bufs=1, space="PSUM") as pgen,
            tc.tile_pool(name="pkt", bufs=2, space="PSUM") as pkt,
            tc.tile_pool(name="ptp", bufs=3, space="PSUM") as ptp,
            tc.tile_pool(name="pout", bufs=2, space="PSUM") as pout,
        ):
            # --- input DMAs on both HWDGE queues (sync + scalar) ---
            cb = cpool.tile([128, NCB], u8, tag="cb")
            nc.scalar.dma_start(cb[:], cb_d)
            xcm = []
            for i, n in enumerate(XCHUNKS):
                t = cpool.tile([C, n], bf16, tag=f"xcm{i}", name=f"xcm{i}")
                nc.sync.dma_start(t[:], xcm_d[i])
                xcm.append(t)
            xtp = []
            for i, n in ((0, 10), (1, 9)):
                t = cpool.tile([128, n * 128], bf16, tag=f"xtp{i}", name=f"xtp{i}")
                nc.scalar.dma_start(t[:], xtp_d[i])
                xtp.append(t)

            w1sT = cb[:, OFF_W1:OFF_W1 + 64].bitcast(bf16)       # [128, 32]
            ident = cb[:, OFF_ID:OFF_ID + 256].bitcast(bf16)     # [128, 128]
            idxt2 = cb[:, OFF_I2:OFF_I2 + 200].bitcast(i16)      # [128, 100]
            b1f = cb[0:32, OFF_B1:OFF_B1 + 4].bitcast(f32)       # [32, 1]
            w2b = cb[0:33, OFF_W2:OFF_W2 + 98].bitcast(bf16)     # [33, 49]

            def xtp_tile(kb):
                return (xtp[0][:, kb * 128:(kb + 1) * 128] if kb < 10
                        else xtp[1][:, (kb - 10) * 128:(kb - 9) * 128])

            outsb = cpool.tile([C, P], bf16, tag="outsb")
            fb = cpool.tile([33, P], bf16, tag="fb")
            # ones row of fb, written as packed pairs of bf16(1.0)
            nc.vector.memset(fb[32:33, :].bitcast(u32), 0x3F803F80)
            kermT = cpool.tile([128, 16 * NO], bf16, tag="kermT")

            # ---- kernel generation (pixel-major kermT[p, o]) interleaved
            # with the GPSIMD banded-matrix scatters; a tiny first chunk gets
            # the scatter chain started as early as possible ----
            a2v = [None] * 16
            CHUNK_TILES = ((0,), (1, 2, 3), (4, 5, 6, 7), (8, 9, 10, 11),
                           (12, 13, 14, 15))
            CHUNK_BATCHES = (((0,),), ((1,), (2, 3)), ((4, 5), (6, 7)),
                             ((8, 9), (10, 11)), ((12, 13), (14, 15)))
            off = 0
            for ci, n in enumerate(XCHUNKS):
                fsl = slice(off, off + n)
                off += n
                f1 = pgen.tile([32, 512], f32, tag="f1")
                nc.tensor.matmul(f1[:, 0:n], w1sT, xcm[ci][:],
                                 start=True, stop=True)
                nc.scalar.activation(
                    fb[0:32, fsl], f1[:, 0:n],
                    mybir.ActivationFunctionType.Relu, bias=b1f,
                )
                for t in CHUNK_TILES[ci]:
                    kt = pkt.tile([128, 512], f32, tag="kt")
                    nc.tensor.matmul(
                        kt[:, 0:49], fb[:, 128 * t:128 * (t + 1)], w2b,
                        start=True, stop=True,
                    )
                    nc.vector.tensor_copy(kermT[:, t * NO:t * NO + 49],
                                          kt[:, 0:49])
                for mbs_b in CHUNK_BATCHES[ci]:
                    k0 = mbs_b[0]
                    nb = len(mbs_b)
                    ab = cpool.tile([128, nb * AWIN], bf16, name=f"a2b{k0}",
                                    tag=f"a2b{k0}")
                    nc.gpsimd.local_scatter(
                        ab[:], kermT[:, k0 * NO:(k0 + nb) * NO],
                        idxt2[:, 0:nb * NO],
                        channels=128, num_elems=nb * AWIN, num_idxs=nb * NO,
                    )
                    for j, mb in enumerate(mbs_b):
                        a2v[mb] = ab[:, j * AWIN:(j + 1) * AWIN]

            # ---- transpose blocks into strips + kb-major matmuls,
            # lagged one iteration so TensorE reaches each matmul only after
            # its strip copy has had a full iteration to complete ----
            po = {}
            strips = [None] * NQT

            def emit_mms(kb):
                base = 128 * (kb - 3)
                for g in range(max(0, (kb - 3) // 4), min(3, kb // 4) + 1):
                    first = g not in po
                    if first:
                        po[g] = pout.tile([C, 512], f32, name=f"po{g}", tag="po")
                    glo, ghi = 512 * g, 512 * (g + 1)
                    last = kb == min(4 * g + 6, NQT - 1)
                    lo = max(glo, base)
                    hi = min(ghi, 128 * (kb + 1) if kb <= 15 else 128 * kb)
                    if hi > lo:
                        nc.tensor.matmul(
                            po[g][:, lo - glo:hi - glo],
                            xtp_tile(kb),
                            strips[kb][:, lo - base:hi - base],
                            start=first, stop=last, skip_group_check=True,
                        )
                    if last:
                        nc.vector.tensor_copy(
                            outsb[:, glo:glo + 256], po[g][:, 0:256])
                        nc.scalar.copy(
                            outsb[:, glo + 256:ghi], po[g][:, 256:512])
                        eng = nc.sync if g % 2 == 0 else nc.scalar
                        eng.dma_start(out_d[:, glo:ghi], outsb[:, glo:ghi])

            for kb in range(NQT):
                mbs = [kb - j for j in range(3, -1, -1) if 0 <= kb - j <= 15]
                tp = ptp.tile([128, 1024], bf16, tag="tp")
                for mb in mbs:
                    pos = mb - (kb - 3)
                    nc.tensor.transpose(
                        tp[:, pos * 128:(pos + 1) * 128],
                        a2v[mb][:, (kb - mb) * 128:(kb - mb + 1) * 128],
                        ident,
                    )
                strip = cpool.tile([128, AWIN], bf16, name=f"st{kb % 4}",
                                   tag=f"st{kb % 4}")
                strips[kb] = strip
                lo_pos = mbs[0] - (kb - 3)
                hi_pos = mbs[-1] - (kb - 3) + 1
                mid_pos = (lo_pos + hi_pos + 1) // 2
                csl0 = slice(lo_pos * 128, mid_pos * 128)
                csl1 = slice(mid_pos * 128, hi_pos * 128)
                nc.vector.tensor_copy(strip[:, csl0], tp[:, csl0])
                if hi_pos > mid_pos:
                    nc.scalar.copy(strip[:, csl1], tp[:, csl1])
                if kb >= 1:
                    emit_mms(kb - 1)
            emit_mms(NQT - 1)

    nc.compile()
    return nc


def _get_nc():
    if "nc" not in _STATE:
        _STATE["nc"] = _build()
    return _STATE["nc"]


def _make_idx_table():
    p_loc = np.arange(128)[:, None]
    o = np.arange(49)[None, :]
    di = o // 7 - 3
    dj = o % 7 - 3
    w_of = p_loc % 64
    idx = p_loc + 192 + 64 * di + dj
    masked = (w_of + dj < 0) | (w_of + dj >= 64)
    idx = np.where(masked, -1, idx)
    tab = np.full((128, NO), -1, dtype=np.int16)
    tab[:, :49] = idx.astype(np.int16)
    return tab


def _host_prep(x, w1, b1, bn_gamma, bn_beta, bn_mean, bn_var, w2, b2):
    x = np.asarray(x, dtype=np.float32)
    scale = np.asarray(bn_gamma) / np.sqrt(np.asarray(bn_var) + EPS)
    w1s = (np.asarray(w1) * scale[:, None]).astype(np.float32)
    b1f = (np.asarray(b1) * scale + np.asarray(bn_beta)
           - np.asarray(bn_mean) * scale).astype(np.float32)
    w1sT = np.ascontiguousarray(w1s.T).astype(BF16)            # [128, 32]
    w2b = np.vstack([np.asarray(w2, np.float32).T,
                     np.asarray(b2, np.float32)[None, :]]).astype(BF16)  # [33, 49]
    idxt = _make_idx_table()                                   # [128, 50] i16
    ident = np.eye(128, dtype=np.float32).astype(BF16)

    cb = np.zeros((128, NCB), np.uint8)
    cb[:, OFF_W1:OFF_W1 + 64] = np.ascontiguousarray(w1sT).view(np.uint8)
    cb[:, OFF_ID:OFF_ID + 256] = np.ascontiguousarray(ident).view(np.uint8)
    idxt2 = np.concatenate(
        [np.where(idxt >= 0, idxt + AWIN * j, -1) for j in range(2)],
        axis=1).astype(np.int16)
    cb[:, OFF_I2:OFF_I2 + 200] = idxt2.view(np.uint8)
    cb[0:32, OFF_B1:OFF_B1 + 4] = np.ascontiguousarray(
        b1f[:, None]).view(np.uint8)
    cb[0:33, OFF_W2:OFF_W2 + 98] = np.ascontiguousarray(w2b).view(np.uint8)

    in_maps = []
    for core in range(8):
        b, half = divmod(core, 2)
        h0 = HH * half
        xcm = np.ascontiguousarray(
            x[b, :, h0:h0 + HH, :].reshape(C, P)).astype(BF16)
        # q-space: rows h0-3 .. h0+35 (zeros outside the image)
        xe = np.zeros((C, HH + 6, W), dtype=np.float32)
        lo = max(0, h0 - 3)
        hi = min(H, h0 + HH + 3)
        xe[:, lo - (h0 - 3):hi - (h0 - 3), :] = x[b, :, lo:hi, :]
        xq = xe.reshape(C, NQT * 128).T                        # [2432, 128]
        xtp = np.ascontiguousarray(
            xq.reshape(NQT, 128, 128).transpose(1, 0, 2).reshape(128, NQT * 128)
        ).astype(BF16)
        bnds = np.cumsum((0, 128, 384, 512, 512, 512))
        im = {f"xcm{i}": xcm[:, bnds[i]:bnds[i + 1]] for i in range(5)}
        im.update({"xtp0": xtp[:, :10 * 128], "xtp1": xtp[:, 10 * 128:],
                   "cb": cb})
        in_maps.append(im)
    return in_maps


def run(inputs: dict, trace: bool = False):
    from concourse.bass_utils import run_bass_kernel_spmd

    nc = _get_nc()
    in_maps = _host_prep(**inputs)
    res = run_bass_kernel_spmd(
        nc, in_maps, core_ids=list(range(8)), trace=trace,
    )
    out = np.zeros((B, C, H, W), dtype=np.float32)
    for core in range(8):
        b, half = divmod(core, 2)
        h0 = HH * half
        out[b, :, h0:h0 + HH, :] = (
            res.results[core]["out"].astype(np.float32).reshape(C, HH, W)
        )
    return out, res


def kernel(**inputs) -> np.ndarray:
    out, _ = run(inputs, trace=False)
    return out


# revision 25
# speedup vs baseline: 1.0575x; 1.0136x over previous
# Involution2d (K=7) Trainium2 kernel — 8-core SPMD, batch+spatial sharding.
#
# Sharding: 8 cores = (batch b in 0..3) x (H-half in 0..1); each core owns a
# [128, 32, 64] output block (2048 pixels, p = 64*h + w).
#
# Per-core algorithm (all-TensorE involution via a banded pixel->pixel matrix):
#   1. gen (bf16): 1x1 conv (BN folded) -> ReLU -> 1x1 conv, emitted directly
#      in pixel-major layout kermT[p, o] (16 matmuls of [33,128]^T @ [33,49];
#      bias rides an ones-row in the stationary operand).
#   2. GPSIMD local_scatter per 128-pixel tile mb: place the 49 kernel values
#      of pixel p at column q - 128*mb of A2T[p, :], where q = p + 192 +
#      64*di + dj is the flattened source pixel (38 rows x 64 cols q-space;
#      halo rows from the neighbor core, zeros at image edges). W-edge terms
#      get idx=-1 (skipped), which provably clips the window to 512 columns.
#   3. TensorE transposes each 128x128 block (bf16, 1 cyc/row) into per-q-tile
#      strips strip[kb] = A2[q, p-window], copied PSUM->SBUF in one 512-wide
#      copy per strip (DVE/ScalarE alternating).
#   4. involution: out[c, p] = sum_q xT[q, c] * A2[q, p] as 28 accumulating
#      bf16 matmuls over 512-column PSUM group tiles, kb-major. Only the
#      first matmul of a group uses start=True (clears the bank's
#      has_written bits); later matmuls overwrite on first element touch and
#      accumulate on repeats.
import numpy as np
import ml_dtypes

EPS = 1e-5
KK = 7
C = 128
H = 64
W = 64
B = 4
HH = 32            # rows per core
P = HH * W         # 2048 output pixels per core
NQT = 19           # q tiles: (HH + 6) * W / 128
NO = 50            # offset count padded to even (49 + 1 dummy)
AWIN = 512         # scatter window (4 q-tiles)

# packed-constants byte layout (per partition)
OFF_W1 = 0         # [128, 32] bf16      -> 64 B
OFF_ID = 64        # [128, 128] bf16     -> 256 B (transpose identity)
OFF_I2 = 320       # [128, 100] int16    -> 200 B (2-tile scatter table)
OFF_B1 = 520       # [32, 1] f32         -> 4 B
OFF_W2 = 524       # [33, 49] bf16       -> 98 B
NCB = 628

_STATE = {}

BF16 = ml_dtypes.bfloat16


def _build():
    import concourse.tile as tile
    from concourse import bacc, mybir

    f32 = mybir.dt.float32
    bf16 = mybir.dt.bfloat16
    i16 = mybir.dt.int16
    u8 = mybir.dt.uint8
    u32 = mybir.dt.uint32
    nc = bacc.Bacc("TRN2", target_bir_lowering=False, debug=False)

    XCHUNKS = (128, 384, 512, 512, 512)
    xcm_d = [
        nc.dram_tensor(f"xcm{i}", [C, n], bf16, kind="ExternalInput").ap()
        for i, n in enumerate(XCHUNKS)
    ]
    xtp_d = [
        nc.dram_tensor(f"xtp{i}", [128, n * 128], bf16, kind="ExternalInput").ap()
        for i, n in ((0, 10), (1, 9))
    ]
    cb_d = nc.dram_tensor("cb", [128, NCB], u8, kind="ExternalInput").ap()
    out_d = nc.dram_tensor("out", [C, P], bf16, kind="ExternalOutput").ap()

    with tile.TileContext(nc) as tc:
        with (
            tc.tile_pool(name="consts", bufs=1) as cpool,
            tc.tile_pool(name="pgen", bufs=1, space="PSUM") as pgen,
            tc.tile_pool(name="pkt", bufs=2, space="PSUM") as pkt,
            tc.tile_pool(name="ptp", bufs=3, space="PSUM") as ptp,
            tc.tile_pool(name="pout", bufs=2, space="PSUM") as pout,
        ):
            # --- input DMAs on both HWDGE queues (sync + scalar) ---
            cb = cpool.tile([128, NCB], u8, tag="cb")
            nc.scalar.dma_start(cb[:], cb_d)
            xcm = []
            for i, n in enumerate(XCHUNKS):
                t = cpool.tile([C, n], bf16, tag=f"xcm{i}", name=f"xcm{i}")
                nc.sync.dma_start(t[:], xcm_d[i])
                xcm.append(t)
            xtp = []
            for i, n in ((0, 10), (1, 9)):
                t = cpool.tile([128, n * 128], bf16, tag=f"xtp{i}", name=f"xtp{i}")
                nc.scalar.dma_start(t[:], xtp_d[i])
                xtp.append(t)

            # Prefetch the GPSIMD local_scatter ucode library: the lazy
            # UNLOAD/LOAD otherwise fires right before the first real scatter
            # and its ~3us image DMA lands on the critical path. A 2-element
            # dummy scatter (all idx=-1) hoists the load into the input-DMA
            # window.
            libw = cpool.tile([128, 2], bf16, tag="libw")
            libi = cpool.tile([128, 2], i16, tag="libi")
            nc.vector.memset(libi[:].bitcast(u32), 0xFFFFFFFF)
            nc.gpsimd.local_scatter(libw[:], libw[:], libi[:],
                                    channels=128, num_elems=2, num_idxs=2)

            w1sT = cb[:, OFF_W1:OFF_W1 + 64].bitcast(bf16)       # [128, 32]
            ident = cb[:, OFF_ID:OFF_ID + 256].bitcast(bf16)     # [128, 128]
            idxt2 = cb[:, OFF_I2:OFF_I2 + 200].bitcast(i16)      # [128, 100]
            b1f = cb[0:32, OFF_B1:OFF_B1 + 4].bitcast(f32)       # [32, 1]
            w2b = cb[0:33, OFF_W2:OFF_W2 + 98].bitcast(bf16)     # [33, 49]

            def xtp_tile(kb):
                return (xtp[0][:, kb * 128:(kb + 1) * 128] if kb < 10
                        else xtp[1][:, (kb - 10) * 128:(kb - 9) * 128])

            outsb = cpool.tile([C, P], bf16, tag="outsb")
            fb = cpool.tile([33, P], bf16, tag="fb")
            # ones row of fb, written as packed pairs of bf16(1.0)
            nc.vector.memset(fb[32:33, :].bitcast(u32), 0x3F803F80)
            kermT = cpool.tile([128, 16 * NO], bf16, tag="kermT")

            # ---- kernel generation (pixel-major kermT[p, o]) interleaved
            # with the GPSIMD banded-matrix scatters; a tiny first chunk gets
            # the scatter chain started as early as possible ----
            a2v = [None] * 16
            CHUNK_TILES = ((0,), (1, 2, 3), (4, 5, 6, 7), (8, 9, 10, 11),
                           (12, 13, 14, 15))
            CHUNK_BATCHES = (((0,),), ((1,), (2, 3)), ((4, 5), (6, 7)),
                             ((8, 9), (10, 11)), ((12, 13), (14, 15)))
            off = 0
            for ci, n in enumerate(XCHUNKS):
                fsl = slice(off, off + n)
                off += n
                f1 = pgen.tile([32, 512], f32, tag="f1")
                nc.tensor.matmul(f1[:, 0:n], w1sT, xcm[ci][:],
                                 start=True, stop=True)
                nc.scalar.activation(
                    fb[0:32, fsl], f1[:, 0:n],
                    mybir.ActivationFunctionType.Relu, bias=b1f,
                )
                for t in CHUNK_TILES[ci]:
                    kt = pkt.tile([128, 512], f32, tag="kt")
                    nc.tensor.matmul(
                        kt[:, 0:49], fb[:, 128 * t:128 * (t + 1)], w2b,
                        start=True, stop=True,
                    )
                    nc.vector.tensor_copy(kermT[:, t * NO:t * NO + 49],
                                          kt[:, 0:49])
                for mbs_b in CHUNK_BATCHES[ci]:
                    k0 = mbs_b[0]
                    nb = len(mbs_b)
                    ab = cpool.tile([128, nb * AWIN], bf16, name=f"a2b{k0}",
                                    tag=f"a2b{k0}")
                    nc.gpsimd.local_scatter(
                        ab[:], kermT[:, k0 * NO:(k0 + nb) * NO],
                        idxt2[:, 0:nb * NO],
                        channels=128, num_elems=nb * AWIN, num_idxs=nb * NO,
                    )
                    for j, mb in enumerate(mbs_b):
                        a2v[mb] = ab[:, j * AWIN:(j + 1) * AWIN]

            # ---- transpose blocks into strips + kb-major matmuls,
            # lagged one iteration so TensorE reaches each matmul only after
            # its strip copy has had a full iteration to complete ----
            po = {}
            strips = [None] * NQT

            def emit_mms(kb):
                base = 128 * (kb - 3)
                for g in range(max(0, (kb - 3) // 4), min(3, kb // 4) + 1):
                    first = g not in po
                    if first:
                        po[g] = pout.tile([C, 512], f32, name=f"po{g}", tag="po")
                    glo, ghi = 512 * g, 512 * (g + 1)
                    last = kb == min(4 * g + 6, NQT - 1)
                    lo = max(glo, base)
                    hi = min(ghi, 128 * (kb + 1) if kb <= 15 else 128 * kb)
                    if hi > lo:
                        nc.tensor.matmul(
                            po[g][:, lo - glo:hi - glo],
                            xtp_tile(kb),
                            strips[kb][:, lo - base:hi - base],
                            start=first, stop=last, skip_group_check=True,
                        )
                    if last:
                        nc.vector.tensor_copy(
                            outsb[:, glo:glo + 256], po[g][:, 0:256])
                        nc.scalar.copy(
                            outsb[:, glo + 256:ghi], po[g][:, 256:512])
                        eng = nc.sync if g % 2 == 0 else nc.scalar
                        eng.dma_start(out_d[:, glo:ghi], outsb[:, glo:ghi])

            for kb in range(NQT):
                mbs = [kb - j for j in range(3, -1, -1) if 0 <= kb - j <= 15]
                tp = ptp.tile([128, 1024], bf16, tag="tp")
                for mb in mbs:
                    pos = mb - (kb - 3)
                    nc.tensor.transpose(
                        tp[:, pos * 128:(pos + 1) * 128],
                        a2v[mb][:, (kb - mb) * 128:(kb - mb + 1) * 128],
                        ident,
                    )
                strip = cpool.tile([128, AWIN], bf16, name=f"st{kb % 4}",
                                   tag=f"st{kb % 4}")
                strips[kb] = strip
                lo_pos = mbs[0] - (kb - 3)
                hi_pos = mbs[-1] - (kb - 3) + 1
                mid_pos = (lo_pos + hi_pos + 1) // 2
                csl0 = slice(lo_pos * 128, mid_pos * 128)
                csl1 = slice(mid_pos * 128, hi_pos * 128)
                nc.vector.tensor_copy(strip[:, csl0], tp[:, csl0])
                if hi_pos > mid_pos:
                    nc.scalar.copy(strip[:, csl1], tp[:, csl1])
                if kb >= 1:
                    emit_mms(kb - 1)
            emit_mms(NQT - 1)

    nc.compile()
    return nc


def _get_nc():
    if "nc" not in _STATE:
        _STATE["nc"] = _build()
    return _STATE["nc"]


def _make_idx_table():
    p_loc = np.arange(128)[:, None]
    o = np.arange(49)[None, :]
    di = o // 7 - 3
    dj = o % 7 - 3
    w_of = p_loc % 64
    idx = p_loc + 192 + 64 * di + dj
    masked = (w_of + dj < 0) | (w_of + dj >= 64)
    idx = np.where(masked, -1, idx)
    tab = np.full((128, NO), -1, dtype=np.int16)
    tab[:, :49] = idx.astype(np.int16)
    return tab


def _host_prep(x, w1, b1, bn_gamma, bn_beta, bn_mean, bn_var, w2, b2):
    x = np.asarray(x, dtype=np.float32)
    scale = np.asarray(bn_gamma) / np.sqrt(np.asarray(bn_var) + EPS)
    w1s = (np.asarray(w1) * scale[:, None]).astype(np.float32)
    b1f = (np.asarray(b1) * scale + np.asarray(bn_beta)
           - np.asarray(bn_mean) * scale).astype(np.float32)
    w1sT = np.ascontiguousarray(w1s.T).astype(BF16)            # [128, 32]
    w2b = np.vstack([np.asarray(w2, np.float32).T,
                     np.asarray(b2, np.float32)[None, :]]).astype(BF16)  # [33, 49]
    idxt = _make_idx_table()                                   # [128, 50] i16
    ident = np.eye(128, dtype=np.float32).astype(BF16)

    cb = np.zeros((128, NCB), np.uint8)
    cb[:, OFF_W1:OFF_W1 + 64] = np.ascontiguousarray(w1sT).view(np.uint8)
    cb[:, OFF_ID:OFF_ID + 256] = np.ascontiguousarray(ident).view(np.uint8)
    idxt2 = np.concatenate(
        [np.where(idxt >= 0, idxt + AWIN * j, -1) for j in range(2)],
        axis=1).astype(np.int16)
    cb[:, OFF_I2:OFF_I2 + 200] = idxt2.view(np.uint8)
    cb[0:32, OFF_B1:OFF_B1 + 4] = np.ascontiguousarray(
        b1f[:, None]).view(np.uint8)
    cb[0:33, OFF_W2:OFF_W2 + 98] = np.ascontiguousarray(w2b).view(np.uint8)

    in_maps = []
    for core in range(8):
        b, half = divmod(core, 2)
        h0 = HH * half
        xcm = np.ascontiguousarray(
            x[b, :, h0:h0 + HH, :].reshape(C, P)).astype(BF16)
        # q-space: rows h0-3 .. h0+35 (zeros outside the image)
        xe = np.zeros((C, HH + 6, W), dtype=np.float32)
        lo = max(0, h0 - 3)
        hi = min(H, h0 + HH + 3)
        xe[:, lo - (h0 - 3):hi - (h0 - 3), :] = x[b, :, lo:hi, :]
        xq = xe.reshape(C, NQT * 128).T                        # [2432, 128]
        xtp = np.ascontiguousarray(
            xq.reshape(NQT, 128, 128).transpose(1, 0, 2).reshape(128, NQT * 128)
        ).astype(BF16)
        bnds = np.cumsum((0, 128, 384, 512, 512, 512))
        im = {f"xcm{i}": xcm[:, bnds[i]:bnds[i + 1]] for i in range(5)}
        im.update({"xtp0": xtp[:, :10 * 128], "xtp1": xtp[:, 10 * 128:],
                   "cb": cb})
        in_maps.append(im)
    return in_maps


def run(inputs: dict, trace: bool = False):
    from concourse.bass_utils import run_bass_kernel_spmd

    nc = _get_nc()
    in_maps = _host_prep(**inputs)
    res = run_bass_kernel_spmd(
        nc, in_maps, core_ids=list(range(8)), trace=trace,
    )
    out = np.zeros((B, C, H, W), dtype=np.float32)
    for core in range(8):
        b, half = divmod(core, 2)
        h0 = HH * half
        out[b, :, h0:h0 + HH, :] = (
            res.results[core]["out"].astype(np.float32).reshape(C, HH, W)
        )
    return out, res


def kernel(**inputs) -> np.ndarray:
    out, _ = run(inputs, trace=False)
    return out


# revision 27
# speedup vs baseline: 1.0726x; 1.0142x over previous
# Involution2d (K=7) Trainium2 kernel — 8-core SPMD, batch+spatial sharding.
#
# Sharding: 8 cores = (batch b in 0..3) x (H-half in 0..1); each core owns a
# [128, 32, 64] output block (2048 pixels, p = 64*h + w).
#
# Per-core algorithm (all-TensorE involution via a banded pixel->pixel matrix):
#   1. gen (bf16): 1x1 conv (BN folded) -> ReLU -> 1x1 conv, emitted directly
#      in pixel-major layout kermT[p, o] (16 matmuls of [33,128]^T @ [33,49];
#      bias rides an ones-row in the stationary operand).
#   2. GPSIMD local_scatter per 128-pixel tile mb: place the 49 kernel values
#      of pixel p at column q - 128*mb of A2T[p, :], where q = p + 192 +
#      64*di + dj is the flattened source pixel (38 rows x 64 cols q-space;
#      halo rows from the neighbor core, zeros at image edges). W-edge terms
#      get idx=-1 (skipped), which provably clips the window to 512 columns.
#   3. TensorE transposes each 128x128 block (bf16, 1 cyc/row) into per-q-tile
#      strips strip[kb] = A2[q, p-window], copied PSUM->SBUF in one 512-wide
#      copy per strip (DVE/ScalarE alternating).
#   4. involution: out[c, p] = sum_q xT[q, c] * A2[q, p] as 28 accumulating
#      bf16 matmuls over 512-column PSUM group tiles, kb-major. Only the
#      first matmul of a group uses start=True (clears the bank's
#      has_written bits); later matmuls overwrite on first element touch and
#      accumulate on repeats.
import numpy as np
import ml_dtypes

EPS = 1e-5
KK = 7
C = 128
H = 64
W = 64
B = 4
HH = 32            # rows per core
P = HH * W         # 2048 output pixels per core
NQT = 19           # q tiles: (HH + 6) * W / 128
NO = 50            # offset count padded to even (49 + 1 dummy)
AWIN = 512         # scatter window (4 q-tiles)

# packed-constants byte layout (per partition)
OFF_W1 = 0         # [128, 32] bf16      -> 64 B
OFF_ID = 64        # [128, 128] bf16     -> 256 B (transpose identity)
OFF_I2 = 320       # [128, 100] int16    -> 200 B (2-tile scatter table)
OFF_B1 = 520       # [32, 1] f32         -> 4 B
OFF_W2 = 524       # [33, 49] bf16       -> 98 B
NCB = 628

_STATE = {}

BF16 = ml_dtypes.bfloat16


def _build():
    import concourse.tile as tile
    from concourse import bacc, mybir

    f32 = mybir.dt.float32
    bf16 = mybir.dt.bfloat16
    i16 = mybir.dt.int16
    u8 = mybir.dt.uint8
    u32 = mybir.dt.uint32
    nc = bacc.Bacc("TRN2", target_bir_lowering=False, debug=False)

    XCHUNKS = (512, 512, 512, 512)
    xcm_d = [
        nc.dram_tensor(f"xcm{i}", [C, n], bf16, kind="ExternalInput").ap()
        for i, n in enumerate(XCHUNKS)
    ]
    xtp_d = [
        nc.dram_tensor(f"xtp{i}", [128, n * 128], bf16, kind="ExternalInput").ap()
        for i, n in ((0, 10), (1, 9))
    ]
    cb_d = nc.dram_tensor("cb", [128, NCB], u8, kind="ExternalInput").ap()
    out_d = nc.dram_tensor("out", [C, P], bf16, kind="ExternalOutput").ap()

    with tile.TileContext(nc) as tc:
        with (
            tc.tile_pool(name="consts", bufs=1) as cpool,
            tc.tile_pool(name="pgen", bufs=1, space="PSUM") as pgen,
            tc.tile_pool(name="pkt", bufs=2, space="PSUM") as pkt,
            tc.tile_pool(name="ptp", bufs=3, space="PSUM") as ptp,
            tc.tile_pool(name="pout", bufs=2, space="PSUM") as pout,
        ):
            # --- input DMAs on both HWDGE queues (sync + scalar) ---
            cb = cpool.tile([128, NCB], u8, tag="cb")
            nc.scalar.dma_start(cb[:], cb_d)
            xcm = []
            for i, n in enumerate(XCHUNKS):
                t = cpool.tile([C, n], bf16, tag=f"xcm{i}", name=f"xcm{i}")
                nc.sync.dma_start(t[:], xcm_d[i])
                xcm.append(t)
            xtp = []
            for i, n in ((0, 10), (1, 9)):
                t = cpool.tile([128, n * 128], bf16, tag=f"xtp{i}", name=f"xtp{i}")
                nc.scalar.dma_start(t[:], xtp_d[i])
                xtp.append(t)

            # Prefetch the GPSIMD local_scatter ucode library: the lazy
            # UNLOAD/LOAD otherwise fires right before the first real scatter
            # and its ~3us image DMA lands on the critical path. A 2-element
            # dummy scatter (all idx=-1) hoists the load into the input-DMA
            # window.
            libw = cpool.tile([128, 2], bf16, tag="libw")
            libi = cpool.tile([128, 2], i16, tag="libi")
            nc.vector.memset(libi[:].bitcast(u32), 0xFFFFFFFF)
            nc.gpsimd.local_scatter(libw[:], libw[:], libi[:],
                                    channels=128, num_elems=2, num_idxs=2)

            w1sT = cb[:, OFF_W1:OFF_W1 + 64].bitcast(bf16)       # [128, 32]
            ident = cb[:, OFF_ID:OFF_ID + 256].bitcast(bf16)     # [128, 128]
            idxt2 = cb[:, OFF_I2:OFF_I2 + 200].bitcast(i16)      # [128, 100]
            b1f = cb[0:32, OFF_B1:OFF_B1 + 4].bitcast(f32)       # [32, 1]
            w2b = cb[0:33, OFF_W2:OFF_W2 + 98].bitcast(bf16)     # [33, 49]

            def xtp_tile(kb):
                return (xtp[0][:, kb * 128:(kb + 1) * 128] if kb < 10
                        else xtp[1][:, (kb - 10) * 128:(kb - 9) * 128])

            outsb = cpool.tile([C, P], bf16, tag="outsb")
            fb = cpool.tile([33, P], bf16, tag="fb")
            # ones row of fb, written as packed pairs of bf16(1.0)
            nc.vector.memset(fb[32:33, :].bitcast(u32), 0x3F803F80)
            kermT = cpool.tile([128, 16 * NO], bf16, tag="kermT")

            # ---- kernel generation (pixel-major kermT[p, o]) interleaved
            # with the GPSIMD banded-matrix scatters; a tiny first chunk gets
            # the scatter chain started as early as possible ----
            a2v = [None] * 16
            CHUNK_TILES = ((0, 1, 2, 3), (4, 5, 6, 7), (8, 9, 10, 11),
                           (12, 13, 14, 15))
            CHUNK_BATCHES = (((0, 1), (2, 3)), ((4, 5), (6, 7)),
                             ((8, 9), (10, 11)), ((12, 13), (14, 15)))
            off = 0
            for ci, n in enumerate(XCHUNKS):
                fsl = slice(off, off + n)
                off += n
                f1 = pgen.tile([32, 512], f32, tag="f1")
                nc.tensor.matmul(f1[:, 0:n], w1sT, xcm[ci][:],
                                 start=True, stop=True)
                nc.scalar.activation(
                    fb[0:32, fsl], f1[:, 0:n],
                    mybir.ActivationFunctionType.Relu, bias=b1f,
                )
                for t in CHUNK_TILES[ci]:
                    kt = pkt.tile([128, 512], f32, tag="kt")
                    nc.tensor.matmul(
                        kt[:, 0:49], fb[:, 128 * t:128 * (t + 1)], w2b,
                        start=True, stop=True,
                    )
                    nc.vector.tensor_copy(kermT[:, t * NO:t * NO + 49],
                                          kt[:, 0:49])
                for mbs_b in CHUNK_BATCHES[ci]:
                    k0 = mbs_b[0]
                    nb = len(mbs_b)
                    ab = cpool.tile([128, nb * AWIN], bf16, name=f"a2b{k0}",
                                    tag=f"a2b{k0}")
                    nc.gpsimd.local_scatter(
                        ab[:], kermT[:, k0 * NO:(k0 + nb) * NO],
                        idxt2[:, 0:nb * NO],
                        channels=128, num_elems=nb * AWIN, num_idxs=nb * NO,
                    )
                    for j, mb in enumerate(mbs_b):
                        a2v[mb] = ab[:, j * AWIN:(j + 1) * AWIN]

            # ---- transpose blocks into strips + kb-major matmuls,
            # lagged one iteration so TensorE reaches each matmul only after
            # its strip copy has had a full iteration to complete ----
            po = {}
            strips = [None] * NQT

            def emit_mms(kb):
                base = 128 * (kb - 3)
                for g in range(max(0, (kb - 3) // 4), min(3, kb // 4) + 1):
                    first = g not in po
                    if first:
                        po[g] = pout.tile([C, 512], f32, name=f"po{g}", tag="po")
                    glo, ghi = 512 * g, 512 * (g + 1)
                    last = kb == min(4 * g + 6, NQT - 1)
                    lo = max(glo, base)
                    hi = min(ghi, 128 * (kb + 1) if kb <= 15 else 128 * kb)
                    if hi > lo:
                        nc.tensor.matmul(
                            po[g][:, lo - glo:hi - glo],
                            xtp_tile(kb),
                            strips[kb][:, lo - base:hi - base],
                            start=first, stop=last, skip_group_check=True,
                        )
                    if last:
                        nc.vector.tensor_copy(
                            outsb[:, glo:glo + 256], po[g][:, 0:256])
                        nc.scalar.copy(
                            outsb[:, glo + 256:ghi], po[g][:, 256:512])
                        eng = nc.sync if g % 2 == 0 else nc.scalar
                        eng.dma_start(out_d[:, glo:ghi], outsb[:, glo:ghi])

            for kb in range(NQT):
                mbs = [kb - j for j in range(3, -1, -1) if 0 <= kb - j <= 15]
                tp = ptp.tile([128, 1024], bf16, tag="tp")
                for mb in mbs:
                    pos = mb - (kb - 3)
                    nc.tensor.transpose(
                        tp[:, pos * 128:(pos + 1) * 128],
                        a2v[mb][:, (kb - mb) * 128:(kb - mb + 1) * 128],
                        ident,
                    )
                strip = cpool.tile([128, AWIN], bf16, name=f"st{kb % 4}",
                                   tag=f"st{kb % 4}")
                strips[kb] = strip
                lo_pos = mbs[0] - (kb - 3)
                hi_pos = mbs[-1] - (kb - 3) + 1
                mid_pos = (lo_pos + hi_pos + 1) // 2
                csl0 = slice(lo_pos * 128, mid_pos * 128)
                csl1 = slice(mid_pos * 128, hi_pos * 128)
                nc.vector.tensor_copy(strip[:, csl0], tp[:, csl0])
                if hi_pos > mid_pos:
                    nc.scalar.copy(strip[:, csl1], tp[:, csl1])
                if kb >= 1:
                    emit_mms(kb - 1)
            emit_mms(NQT - 1)

    nc.compile()
    return nc


def _get_nc():
    if "nc" not in _STATE:
        _STATE["nc"] = _build()
    return _STATE["nc"]


def _make_idx_table():
    p_loc = np.arange(128)[:, None]
    o = np.arange(49)[None, :]
    di = o // 7 - 3
    dj = o % 7 - 3
    w_of = p_loc % 64
    idx = p_loc + 192 + 64 * di + dj
    masked = (w_of + dj < 0) | (w_of + dj >= 64)
    idx = np.where(masked, -1, idx)
    tab = np.full((128, NO), -1, dtype=np.int16)
    tab[:, :49] = idx.astype(np.int16)
    return tab


def _host_prep(x, w1, b1, bn_gamma, bn_beta, bn_mean, bn_var, w2, b2):
    x = np.asarray(x, dtype=np.float32)
    scale = np.asarray(bn_gamma) / np.sqrt(np.asarray(bn_var) + EPS)
    w1s = (np.asarray(w1) * scale[:, None]).astype(np.float32)
    b1f = (np.asarray(b1) * scale + np.asarray(bn_beta)
           - np.asarray(bn_mean) * scale).astype(np.float32)
    w1sT = np.ascontiguousarray(w1s.T).astype(BF16)            # [128, 32]
    w2b = np.vstack([np.asarray(w2, np.float32).T,
                     np.asarray(b2, np.float32)[None, :]]).astype(BF16)  # [33, 49]
    idxt = _make_idx_table()                                   # [128, 50] i16
    ident = np.eye(128, dtype=np.float32).astype(BF16)

    cb = np.zeros((128, NCB), np.uint8)
    cb[:, OFF_W1:OFF_W1 + 64] = np.ascontiguousarray(w1sT).view(np.uint8)
    cb[:, OFF_ID:OFF_ID + 256] = np.ascontiguousarray(ident).view(np.uint8)
    idxt2 = np.concatenate(
        [np.where(idxt >= 0, idxt + AWIN * j, -1) for j in range(2)],
        axis=1).astype(np.int16)
    cb[:, OFF_I2:OFF_I2 + 200] = idxt2.view(np.uint8)
    cb[0:32, OFF_B1:OFF_B1 + 4] = np.ascontiguousarray(
        b1f[:, None]).view(np.uint8)
    cb[0:33, OFF_W2:OFF_W2 + 98] = np.ascontiguousarray(w2b).view(np.uint8)

    in_maps = []
    for core in range(8):
        b, half = divmod(core, 2)
        h0 = HH * half
        xcm = np.ascontiguousarray(
            x[b, :, h0:h0 + HH, :].reshape(C, P)).astype(BF16)
        # q-space: rows h0-3 .. h0+35 (zeros outside the image)
        xe = np.zeros((C, HH + 6, W), dtype=np.float32)
        lo = max(0, h0 - 3)
        hi = min(H, h0 + HH + 3)
        xe[:, lo - (h0 - 3):hi - (h0 - 3), :] = x[b, :, lo:hi, :]
        xq = xe.reshape(C, NQT * 128).T                        # [2432, 128]
        xtp = np.ascontiguousarray(
            xq.reshape(NQT, 128, 128).transpose(1, 0, 2).reshape(128, NQT * 128)
        ).astype(BF16)
        im = {f"xcm{i}": xcm[:, 512 * i:512 * (i + 1)] for i in range(4)}
        im.update({"xtp0": xtp[:, :10 * 128], "xtp1": xtp[:, 10 * 128:],
                   "cb": cb})
        in_maps.append(im)
    return in_maps


def run(inputs: dict, trace: bool = False):
    from concourse.bass_utils import run_bass_kernel_spmd

    nc = _get_nc()
    in_maps = _host_prep(**inputs)
    res = run_bass_kernel_spmd(
        nc, in_maps, core_ids=list(range(8)), trace=trace,
    )
    out = np.zeros((B, C, H, W), dtype=np.float32)
    for core in range(8):
        b, half = divmod(core, 2)
        h0 = HH * half
        out[b, :, h0:h0 + HH, :] = (
            res.results[core]["out"].astype(np.float32).reshape(C, HH, W)
        )
    return out, res


def kernel(**inputs) -> np.ndarray:
    out, _ = run(inputs, trace=False)
    return out
